# revision 4
# baseline (speedup 1.0000x reference)
"""Trainium2 Bass kernel for LGA histogram binning (nn_LGA_49331994362180).

Per (b,n) center with K=32 neighbors: bin each neighbor into one of BETA=6
sphere directions by argmax of dot(rel, sphere_dir) (argmax is invariant to
the positive per-neighbor normalization, so the sqrt/normalize is skipped),
then per-bin counts / direction sums / feature sums with the reference's
normalizations.

Sharding: data-parallel over flattened (B*N) centers across 8 cores.

Device layout: centers are processed in groups of 4; SBUF partition
p = (center%4)*32 + k.  Per group one matmul with a block-diagonal one-hot
lhsT [128, 24] contracts K into the 6 bins for 4 centers at once:
  out[6c'+a, j] = sum_k onehot[c',k,a] * dat[c',k,j]
where rhs dat[:, j] packs [knn_x(64) | rel(3) | ones(1)] -> feature sums,
direction sums and counts in one shot.  Groups stack 4-vertical (PSUM
partition offset 32v via tile_position) x 4-horizontal per PSUM bank.
Post-processing is per-partition only, except the 6-bin sum for
direction_percentage which uses a constant block-ones matmul.
"""

import sys

sys.path.insert(0, "/opt/trn_rl_repo")

from contextlib import ExitStack

import numpy as np

import concourse.bass as bass
import concourse.tile as tile
from concourse import mybir
from concourse.bass_utils import run_bass_kernel_spmd

B, N, K, D = 4, 4096, 32, 64
BETA = 6
ALPHA = 2.0
NCORES = 8
M = B * N                  # 16384 centers
MC = M // NCORES           # 2048 centers per core
J = D + 3 + 1              # 68 packed columns: knn_x | rel | ones
GRP = MC // 4              # 512 groups of 4 centers per core
G = 32                     # groups per tile (128 centers)
NT = GRP // G              # 16 tiles per core
ROWS = 24                  # 4 centers * 6 bins per group-band

op = mybir.AluOpType
f32 = mybir.dt.float32


def _ap(base, off, dims):
    """Free-dim view of a contiguous [128, F] SBUF tile: dims = [(step, count)...]."""
    return bass.AP(
        tensor=base.tensor,
        offset=base.offset + off,
        ap=[list(base.ap[0])] + [[s, c] for s, c in dims],
    )


def _split_waits(nc, maxw=1):
    """walrus CoreV3 rejects >1 sem wait on one instruction; spread excess
    waits over inserted same-engine Drain carriers placed just before."""
    for func in nc.m.functions:
        for block in func.blocks:
            newlist = []
            for inst in block.instructions:
                si = getattr(inst, "sync_info", None)
                if si is not None and len(si.on_wait) > maxw:
                    waits = list(si.on_wait)
                    extra, keep = waits[:-maxw], waits[-maxw:]
                    while extra:
                        chunk, extra = extra[:maxw], extra[maxw:]
                        d = mybir.InstDrain(name=nc.get_next_instruction_name())
                        d.engine = inst.engine
                        d.sync_info = mybir.SyncInfo(on_wait=chunk, on_update=[])
                        newlist.append(d)
                    inst.sync_info = mybir.SyncInfo(
                        on_wait=keep, on_update=list(si.on_update)
                    )
                newlist.append(inst)
            block.instructions[:] = newlist


def build_program():
    nc = bass.Bass()
    data = nc.dram_tensor("data", [128, GRP * J], f32, kind="ExternalInput")
    sphereb = nc.dram_tensor("sphereb", [128, BETA * 3], f32, kind="ExternalInput")
    bin0mask = nc.dram_tensor("bin0mask", [128, 1], f32, kind="ExternalInput")
    bo = nc.dram_tensor("bo", [128, 128], f32, kind="ExternalInput")
    dump = nc.dram_tensor("dump", [NT, 120, 8 * J], f32, kind="ExternalOutput")

    with tile.TileContext(nc) as tc, ExitStack() as ctx:
        consts = ctx.enter_context(tc.tile_pool(name="consts", bufs=1))
        dpool = ctx.enter_context(tc.tile_pool(name="dat", bufs=3))
        work = ctx.enter_context(tc.tile_pool(name="work", bufs=2))
        bpool = ctx.enter_context(tc.tile_pool(name="bs", bufs=3))
        small = ctx.enter_context(tc.tile_pool(name="small", bufs=2))
        psum = ctx.enter_context(tc.tile_pool(name="psum", bufs=2, space="PSUM"))
        psum_s = ctx.enter_context(tc.tile_pool(name="psum_s", bufs=2, space="PSUM"))

        sph = consts.tile([128, BETA * 3], f32)
        nc.sync.dma_start(out=sph, in_=sphereb[:, :])
        b0m = consts.tile([128, 1], f32)
        nc.sync.dma_start(out=b0m, in_=bin0mask[:, :])
        bot = consts.tile([128, 128], f32)
        nc.sync.dma_start(out=bot, in_=bo[:, :])

        for t in range(NT):
            dat = dpool.tile([128, G * J], f32)
            nc.sync.dma_start(out=dat, in_=data[:, t * G * J:(t + 1) * G * J])

            # simm[p, g, a] = sum_s rel[p, g, s] * sphere[a, s]
            tmp = work.tile([128, G * BETA * 3], f32)
            nc.gpsimd.tensor_tensor(
                out=_ap(tmp, 0, [(18, G), (3, BETA), (1, 3)]),
                in0=_ap(dat, D, [(J, G), (0, BETA), (1, 3)]),
                in1=_ap(sph, 0, [(0, G), (3, BETA), (1, 3)]),
                op=op.mult,
            )
            simm = work.tile([128, G * BETA], f32)
            nc.vector.tensor_reduce(
                out=_ap(simm, 0, [(BETA, G), (1, BETA)]),
                in_=_ap(tmp, 0, [(18, G), (3, BETA), (1, 3)]),
                axis=mybir.AxisListType.X,
                op=op.add,
            )
            rmax = small.tile([128, G], f32)
            nc.vector.tensor_reduce(
                out=rmax,
                in_=_ap(simm, 0, [(BETA, G), (1, BETA)]),
                axis=mybir.AxisListType.X,
                op=op.max,
            )
            onehot = work.tile([128, G * BETA], f32)
            nc.vector.tensor_tensor(
                out=_ap(onehot, 0, [(BETA, G), (1, BETA)]),
                in0=_ap(simm, 0, [(BETA, G), (1, BETA)]),
                in1=_ap(rmax, 0, [(1, G), (0, BETA)]),
                op=op.is_ge,
            )

            # block-diagonal lhsT: [128, G*24]; band c' -> cols 6c'..6c'+5
            lz = work.tile([128, G * ROWS], f32)
            nc.gpsimd.memset(lz, 0.0)
            for c in range(4):
                nc.scalar.copy(
                    out=_ap(lz[32 * c:32 * (c + 1)], 6 * c, [(ROWS, G), (1, BETA)]),
                    in_=_ap(onehot[32 * c:32 * (c + 1)], 0, [(BETA, G), (1, BETA)]),
                )

            psA = psum.tile([128, 4 * J], f32, tag="psA")
            psB = psum.tile([128, 4 * J], f32, tag="psB")
            for g in range(G):
                ps = psA if g < 16 else psB
                v = g % 4
                h = (g // 4) % 4
                nc.tensor.matmul(
                    ps[32 * v:32 * v + ROWS, h * J:(h + 1) * J],
                    lz[:, g * ROWS:(g + 1) * ROWS],
                    dat[:, g * J:(g + 1) * J],
                    start=True,
                    stop=True,
                    tile_position=(0, 32 * v),
                )

            bs = bpool.tile([128, 8 * J], f32)
            # zero first, then copy only the 24 valid rows of each 32-band,
            # so uninitialized PSUM never reaches the block-ones matmul
            nc.gpsimd.memset(bs, 0.0)
            for v in range(4):
                nc.scalar.copy(
                    out=bs[32 * v:32 * v + ROWS, 0:4 * J],
                    in_=psA[32 * v:32 * v + ROWS, :],
                )
                nc.scalar.copy(
                    out=bs[32 * v:32 * v + ROWS, 4 * J:8 * J],
                    in_=psB[32 * v:32 * v + ROWS, :],
                )

            # ---- post-processing on bs viewed [128, 8, 68] ----
            ct0 = small.tile([128, 8], f32)   # counts with bin0 -= 1
            nc.vector.tensor_scalar(
                out=ct0, in0=_ap(bs, D + 3, [(J, 8)]),
                scalar1=b0m[:, 0:1], scalar2=None, op0=op.subtract,
            )
            rc = small.tile([128, 8], f32)
            nc.vector.tensor_scalar_add(rc, ct0, 1e-8)
            nc.vector.reciprocal(rc, rc)
            nc.vector.tensor_tensor(
                out=_ap(bs, D, [(J, 8), (1, 3)]),
                in0=_ap(bs, D, [(J, 8), (1, 3)]),
                in1=_ap(rc, 0, [(1, 8), (0, 3)]),
                op=op.mult,
            )
            fs = small.tile([128, 8], f32)
            nc.vector.tensor_reduce(
                out=fs, in_=_ap(bs, 0, [(J, 8), (1, D)]),
                axis=mybir.AxisListType.X, op=op.add,
            )
            rf = small.tile([128, 8], f32)
            nc.vector.tensor_scalar_add(rf, fs, 1e-9)
            nc.vector.reciprocal(rf, rf)
            nc.gpsimd.tensor_tensor(
                out=_ap(bs, 0, [(J, 8), (1, D)]),
                in0=_ap(bs, 0, [(J, 8), (1, D)]),
                in1=_ap(rf, 0, [(1, 8), (0, D)]),
                op=op.mult,
            )
            ctt = small.tile([128, 8], f32)
            nc.vector.tensor_scalar(
                out=ctt, in0=ct0, scalar1=ALPHA, scalar2=None, op0=op.is_gt,
            )
            ctm = small.tile([128, 8], f32)
            nc.vector.tensor_mul(ctm, ct0, ctt)
            ps2 = psum_s.tile([128, 8], f32)
            nc.tensor.matmul(ps2, bot[:, :], ctm, start=True, stop=True)
            rps = small.tile([128, 8], f32)
            nc.vector.tensor_scalar_add(rps, ps2, 1e-8)
            rps2 = small.tile([128, 8], f32)
            nc.vector.reciprocal(rps2, rps)
            nc.vector.tensor_tensor(
                out=_ap(bs, D + 3, [(J, 8)]), in0=ctm, in1=rps2, op=op.mult,
            )

            nc.sync.dma_start(out=dump[t, :, :], in_=bs[0:120, :])

    _split_waits(nc)
    return nc


_PROG = None


def kernel(lc_xyz, lc_x, knn_xyz, knn_x, sphere):
    global _PROG
    lc_xyz = np.asarray(lc_xyz, np.float32)
    knn_xyz = np.asarray(knn_xyz, np.float32)
    knn_x_f = np.asarray(knn_x, np.float32)
    sphere_f = np.asarray(sphere, np.float32)

    rel = knn_xyz - lc_xyz[:, :, None, :]                      # [B,N,K,3]
    dataf = np.empty((M, K, J), np.float32)
    dataf[:, :, :D] = knn_x_f.reshape(M, K, D)
    dataf[:, :, D:D + 3] = rel.reshape(M, K, 3)
    dataf[:, :, D + 3] = 1.0
    # SBUF image: partition p = (center%4)*32 + k, columns = group-major
    img = np.ascontiguousarray(
        dataf.reshape(M // 4, 4, K, J).transpose(1, 2, 0, 3).reshape(128, (M // 4) * J)
    )

    sphereb = np.tile(sphere_f.reshape(1, BETA * 3), (128, 1)).copy()
    r = np.arange(128)
    u = r % 32
    valid = u < ROWS
    bin0mask = (valid & (u % BETA == 0)).astype(np.float32).reshape(128, 1)
    blk = np.where(valid, (r // 32) * 4 + u // BETA, -1)
    bo = ((blk[:, None] == blk[None, :]) & valid[:, None] & valid[None, :]).astype(
        np.float32
    )

    if _PROG is None:
        _PROG = build_program()

    in_maps = []
    for i in range(NCORES):
        sl = np.ascontiguousarray(img[:, i * GRP * J:(i + 1) * GRP * J])
        in_maps.append(
            {"data": sl, "sphereb": sphereb, "bin0mask": bin0mask, "bo": bo}
        )
    res = run_bass_kernel_spmd(_PROG, in_maps, list(range(NCORES)))

    dumps = np.stack([res.results[i]["dump"] for i in range(NCORES)])  # [8,NT,120,544]
    rows = 32 * np.arange(4)[:, None] + np.arange(ROWS)[None, :]       # [4,24]
    d2 = dumps[:, :, rows, :]                                  # [8,NT,4,24,8*J]
    d3 = d2.reshape(NCORES, NT, 4, 4, BETA, 2, 4, J)           # v,c',a,half,h4,col
    # center = t*128 + 64*half + 16*h4 + 4*v + c'
    arr = d3.transpose(0, 1, 5, 6, 2, 3, 4, 7).reshape(M, BETA, J)

    avg_features = np.ascontiguousarray(arr[:, :, :D]).reshape(B, N, BETA, D)
    avg_direction = np.ascontiguousarray(arr[:, :, D:D + 3]).reshape(B, N, BETA, 3)
    direction_percentage = np.ascontiguousarray(arr[:, :, D + 3]).reshape(B, N, BETA)
    k_influence = np.ones((B, N), np.float32)
    return (
        np.asarray(knn_x),
        direction_percentage,
        avg_direction,
        avg_features,
        k_influence,
    )
